# revision 7
# baseline (speedup 1.0000x reference)
"""Trainium2 Bass kernel for nn_MoEConnectionProcessor.

Self-contained: stages/shards the full inputs on host (numpy), runs an SPMD
Bass/Tile kernel on 8 NeuronCores, gathers the full output.

Reference math (per cell, K=26 neighbors, D=32):
  masks by tier (0=local,1=functional,2=distant); masked neighbor means;
  local expert  = tanh([cs, loc_mean] @ W_local + b_local)
  func expert   = (1-z)*cs + z*tanh(agg),  z = sigmoid([cs, agg] @ W_upd + b_upd)
                  agg = masked_mean_k tanh(nb @ W_msg + b_msg)
  dist expert   = 3-step Euler: x += (1/3) tanh([x, agg_d] @ W_cnf + b_cnf)
  gates         = softmax([cs, mean_nb] @ W_g1 + b_g1 -> relu -> @ W_g2 + b_g2)
  out           = sum_t gate_t * expert_t

Device layout strategy per 128-cell tile (cells on SBUF partitions):
  - neighbor data staged natural [cells, (k d)] bf16; DVE StreamTranspose
    gives the d-on-partition operand for the PE matmul with a 4x block
    diagonal W_msg (contraction=32 features x 4 cell subgroups).
  - masked k-sums: DVE broadcast-AP multiplies + PE accumulation matmuls
    (constant identity stationary, 26 accumulating steps).
  - per-cell expert matmuls run in "block-T" layout (features on partitions,
    32-cell blocks) with 4x block-diagonal weights; biases become
    per-partition ACT bias vectors.
"""

import numpy as np
import ml_dtypes
from contextlib import ExitStack

import concourse.bass as bass
import concourse.bacc as bacc
import concourse.tile as tile
import concourse.mybir as mybir

B, K, D, NH = 262144, 26, 32, 32
N_CORES = 8
BS = B // N_CORES  # 32768 cells per core
CT = 128           # cells per tile
N_STEPS = 3
DT_STEP = 1.0 / N_STEPS

dt = mybir.dt
bf16 = ml_dtypes.bfloat16
AF = mybir.ActivationFunctionType
ALU = mybir.AluOpType

# column offsets into the packed weight-constant dram tensor [128, WC_COLS]
_WSLOTS = ["W4msg", "Wl_t", "Wl_b", "Wu_t", "Wu_b", "Wc_t", "Wc_b",
           "Wg1_t", "Wg1_b", "I128"]
WC_COLS = 128 * len(_WSLOTS) + 96  # + Wg2rep [128, 96]
BC_COLS = 8  # f32 bias consts


def _wslot(name):
    return 128 * _WSLOTS.index(name)


def build_program(bs=BS, ct=CT):
    """Builds the per-core Bass program (SPMD; all cores identical)."""
    nt = bs // ct
    nc = bacc.Bacc("TRN2", target_bir_lowering=False, debug=False,
                   num_devices=N_CORES)

    a_nbn = nc.dram_tensor("nbn", [bs, K * D], dt.bfloat16, kind="ExternalInput").ap()
    a_csn = nc.dram_tensor("csn", [bs, D], dt.float32, kind="ExternalInput").ap()
    a_cst = nc.dram_tensor("cst", [128, nt * D], dt.bfloat16, kind="ExternalInput").ap()
    a_msk = nc.dram_tensor("msk", [bs, 80], dt.bfloat16, kind="ExternalInput").ap()
    a_scl = nc.dram_tensor("scl", [bs, 4], dt.float32, kind="ExternalInput").ap()
    a_wc = nc.dram_tensor("wc", [128, WC_COLS], dt.bfloat16, kind="ExternalInput").ap()
    a_bc = nc.dram_tensor("bc", [128, BC_COLS], dt.float32, kind="ExternalInput").ap()
    a_out = nc.dram_tensor("out", [bs, D], dt.float32, kind="ExternalOutput").ap()

    with tile.TileContext(nc) as tc:
        _body(tc, a_nbn, a_csn, a_cst, a_msk, a_scl, a_wc, a_bc, a_out, bs, ct, nt)
    nc.compile()
    return nc


def _body(tc, a_nbn, a_csn, a_cst, a_msk, a_scl, a_wc, a_bc, a_out, bs, ct, nt):
    nc = tc.nc
    FR = K * D  # 832

    with ExitStack() as ctx:
        cpool = ctx.enter_context(tc.tile_pool(name="const", bufs=1))
        pin = ctx.enter_context(tc.tile_pool(name="in", bufs=3))
        psml = ctx.enter_context(tc.tile_pool(name="small", bufs=3))
        pbig = ctx.enter_context(tc.tile_pool(name="big", bufs=2))
        pps_m = ctx.enter_context(tc.tile_pool(name="psm", bufs=2, space="PSUM"))
        pps_s = ctx.enter_context(tc.tile_pool(name="pss", bufs=2, space="PSUM"))
        pps_d = ctx.enter_context(tc.tile_pool(name="psd", bufs=2, space="PSUM"))

        wc = cpool.tile([128, WC_COLS], dt.bfloat16, tag="wc")
        nc.sync.dma_start(wc[:], a_wc)
        bc = cpool.tile([128, BC_COLS], dt.float32, tag="bc")
        nc.sync.dma_start(bc[:], a_bc)

        def W(name):
            return wc[:, _wslot(name): _wslot(name) + 128]

        w2rep = wc[:, 128 * len(_WSLOTS): 128 * len(_WSLOTS) + 96]
        b_msg4 = bc[:, 0:1]
        b_loc4 = bc[:, 1:2]
        b_upd4 = bc[:, 2:3]
        b_cnf4 = bc[:, 3:4]
        b_g14 = bc[:, 4:5]
        bg2rep = bc[:, 5:8]  # [128, 3] replicated b_g2 row

        for t in range(nt):
            r0 = t * ct
            rows = slice(r0, r0 + ct)

            # ---- loads ----
            nb = pin.tile([128, FR], dt.bfloat16, tag="nb")
            nc.sync.dma_start(nb[:], a_nbn[rows, :])
            csn = psml.tile([128, D], dt.float32, tag="csn")
            nc.sync.dma_start(csn[:], a_csn[rows, :])
            cst = psml.tile([128, D], dt.bfloat16, tag="cst")
            nc.sync.dma_start(cst[:], a_cst[:, t * D:(t + 1) * D])
            msk = psml.tile([128, 80], dt.bfloat16, tag="msk")
            nc.sync.dma_start(msk[:], a_msk[rows, :])
            scl = psml.tile([128, 4], dt.float32, tag="scl")
            nc.sync.dma_start(scl[:], a_scl[rows, :])

            nb3 = nb[:].rearrange("p (k d) -> p k d", k=K)

            # ---- transpose for the message matmul ----
            nbT = pbig.tile([128, FR], dt.bfloat16, tag="nbT")
            nc.vector.transpose(nbT[:], nb[:])

            # ---- msgs = tanh(nb @ W_msg + b_msg), transposed layout ----
            ps_m0 = pps_m.tile([128, 416], dt.float32, tag="psm0")
            ps_m1 = pps_m.tile([128, 416], dt.float32, tag="psm1")
            nc.tensor.matmul(ps_m0[:], W("W4msg"), nbT[:, 0:416], start=True, stop=True)
            nc.tensor.matmul(ps_m1[:], W("W4msg"), nbT[:, 416:832], start=True, stop=True)
            msgsT = pbig.tile([128, FR], dt.bfloat16, tag="msgsT")
            nc.scalar.activation(msgsT[:, 0:416], ps_m0[:], AF.Tanh, bias=b_msg4, scale=1.0)
            nc.scalar.activation(msgsT[:, 416:832], ps_m1[:], AF.Tanh, bias=b_msg4, scale=1.0)

            # back to natural layout for the masked k-sum
            msgs_nat = pbig.tile([128, FR], dt.bfloat16, tag="msgsnat")
            nc.vector.transpose(msgs_nat[:], msgsT[:])
            msgs_nat3 = msgs_nat[:].rearrange("p (k d) -> p k d", k=K)

            # ---- masked products (broadcast-AP multiplies) ----
            def bmask(c0):
                return msk[:, c0:c0 + K].unsqueeze(2).to_broadcast((128, K, D))

            prodF = pbig.tile([128, FR], dt.bfloat16, tag="prodF")
            nc.vector.tensor_tensor(
                out=prodF[:].rearrange("p (k d) -> p k d", k=K),
                in0=msgs_nat3, in1=bmask(52), op=ALU.mult)
            prodA = pbig.tile([128, FR], dt.bfloat16, tag="prodA")
            nc.vector.tensor_tensor(
                out=prodA[:].rearrange("p (k d) -> p k d", k=K),
                in0=nb3, in1=bmask(0), op=ALU.mult)
            prodB = pbig.tile([128, FR], dt.bfloat16, tag="prodB")
            nc.vector.tensor_tensor(
                out=prodB[:].rearrange("p (k d) -> p k d", k=K),
                in0=nb3, in1=bmask(26), op=ALU.mult)

            # ---- k-sums via PE accumulation (identity stationary) ----
            # ps_sums columns: S0 @0, A @32, B @64, agg @96
            ps_sums = pps_s.tile([128, 128], dt.float32, tag="sums")
            srcs = [nb3, prodA[:].rearrange("p (k d) -> p k d", k=K),
                    prodB[:].rearrange("p (k d) -> p k d", k=K),
                    prodF[:].rearrange("p (k d) -> p k d", k=K)]
            for j, src in enumerate(srcs):
                for b in range(K):
                    nc.tensor.matmul(ps_sums[:, 32 * j:32 * j + 32], W("I128"),
                                     src[:, b, :], start=(b == 0), stop=(b == K - 1))

            S0 = ps_sums[:, 0:32]
            SA = ps_sums[:, 32:64]
            SB = ps_sums[:, 64:96]
            Sagg = ps_sums[:, 96:128]

            # ---- means (natural, f32) ----
            S0sb = psml.tile([128, D], dt.float32, tag="S0sb")
            nc.vector.tensor_copy(S0sb[:], S0)
            tmp_loc = psml.tile([128, D], dt.float32, tag="tmploc")
            nc.vector.tensor_tensor(out=tmp_loc[:], in0=S0sb[:], in1=SA, op=ALU.subtract)
            mean_loc = psml.tile([128, D], dt.bfloat16, tag="mloc")
            nc.vector.tensor_scalar(out=mean_loc[:], in0=tmp_loc[:],
                                    scalar1=scl[:, 0:1], scalar2=None, op0=ALU.mult)
            mean_dis = psml.tile([128, D], dt.bfloat16, tag="mdis")
            nc.vector.tensor_scalar(out=mean_dis[:], in0=SB,
                                    scalar1=scl[:, 1:2], scalar2=None, op0=ALU.mult)
            S0b16 = psml.tile([128, D], dt.bfloat16, tag="S0b16")
            nc.vector.tensor_copy(S0b16[:], S0sb[:])
            agg16 = psml.tile([128, D], dt.bfloat16, tag="agg16")
            nc.vector.tensor_copy(agg16[:], Sagg)

            # ---- tiny transposes into block-T layout (bf16 operands) ----
            mlT = psml.tile([128, D], dt.bfloat16, tag="mlT")
            nc.vector.transpose(mlT[:], mean_loc[:])
            mdT = psml.tile([128, D], dt.bfloat16, tag="mdT")
            nc.vector.transpose(mdT[:], mean_dis[:])
            mnT = psml.tile([128, D], dt.bfloat16, tag="mnT")
            nc.vector.transpose(mnT[:], S0b16[:])  # 1/K folded into Wg1_b on host
            aggT = psml.tile([128, D], dt.bfloat16, tag="aggT")
            nc.vector.transpose(aggT[:], agg16[:])
            xT = psml.tile([128, D], dt.float32, tag="xT")
            nc.vector.transpose(xT[:], csn[:])

            # ---- experts (block-T, PE + ACT) ----
            ps_dn = pps_d.tile([128, 192], dt.float32, tag="dn")

            nc.tensor.matmul(ps_dn[:, 0:32], W("Wl_t"), cst[:], start=True, stop=False)
            nc.tensor.matmul(ps_dn[:, 0:32], W("Wl_b"), mlT[:], start=False, stop=True)
            localT = psml.tile([128, D], dt.float32, tag="localT")
            nc.scalar.activation(localT[:], ps_dn[:, 0:32], AF.Tanh, bias=b_loc4, scale=1.0)

            nc.tensor.matmul(ps_dn[:, 32:64], W("Wu_t"), cst[:], start=True, stop=False)
            nc.tensor.matmul(ps_dn[:, 32:64], W("Wu_b"), aggT[:], start=False, stop=True)
            zT = psml.tile([128, D], dt.float32, tag="zT")
            nc.scalar.activation(zT[:], ps_dn[:, 32:64], AF.Sigmoid, bias=b_upd4, scale=1.0)

            nc.tensor.matmul(ps_dn[:, 64:96], W("Wg1_t"), cst[:], start=True, stop=False)
            nc.tensor.matmul(ps_dn[:, 64:96], W("Wg1_b"), mnT[:], start=False, stop=True)
            hT = psml.tile([128, D], dt.bfloat16, tag="hT")
            nc.scalar.activation(hT[:], ps_dn[:, 64:96], AF.Relu, bias=b_g14, scale=1.0)

            # CNF Euler steps (x kept f32, bf16 copies feed the PE)
            xcur = xT
            xbf = cst  # step-1 moving operand is exactly csT (bf16)
            for s in range(N_STEPS):
                nc.tensor.matmul(ps_dn[:, 128:160], W("Wc_t"), xbf[:], start=True, stop=False)
                nc.tensor.matmul(ps_dn[:, 128:160], W("Wc_b"), mdT[:], start=False, stop=True)
                vb = psml.tile([128, D], dt.float32, tag=f"vb{s}")
                nc.scalar.activation(vb[:], ps_dn[:, 128:160], AF.Tanh, bias=b_cnf4, scale=1.0)
                xnew = psml.tile([128, D], dt.float32, tag=f"xn{s}")
                nc.vector.scalar_tensor_tensor(out=xnew[:], in0=vb[:], scalar=DT_STEP,
                                               in1=xcur[:], op0=ALU.mult, op1=ALU.add)
                xcur = xnew
                if s < N_STEPS - 1:
                    xb2 = psml.tile([128, D], dt.bfloat16, tag=f"xb{s}")
                    nc.scalar.copy(xb2[:], xnew[:])
                    xbf = xb2

            # ---- gating (natural layout) ----
            h_nat = psml.tile([128, D], dt.bfloat16, tag="hnat")
            nc.vector.transpose(h_nat[:], hT[:])
            lg = psml.tile([128, 4], dt.float32, tag="lg")
            for g in range(3):
                gp = psml.tile([128, D], dt.bfloat16, tag="gp")
                nc.vector.tensor_tensor(out=gp[:], in0=h_nat[:],
                                        in1=w2rep[:, 32 * g:32 * g + 32], op=ALU.mult)
                nc.vector.tensor_reduce(out=lg[:, g:g + 1], in_=gp[:],
                                        axis=mybir.AxisListType.X, op=ALU.add)
            lgb = psml.tile([128, 3], dt.float32, tag="lgb")
            nc.vector.tensor_tensor(out=lgb[:], in0=lg[:, 0:3], in1=bg2rep, op=ALU.add)
            eg = psml.tile([128, 3], dt.float32, tag="eg")
            nc.scalar.activation(eg[:], lgb[:], AF.Exp)
            sg = psml.tile([128, 1], dt.float32, tag="sg")
            nc.vector.tensor_reduce(out=sg[:], in_=eg[:], axis=mybir.AxisListType.X, op=ALU.add)
            rinv = psml.tile([128, 1], dt.float32, tag="rinv")
            nc.vector.reciprocal(rinv[:], sg[:])
            gts = psml.tile([128, 3], dt.float32, tag="gts")
            nc.vector.tensor_scalar(out=gts[:], in0=eg[:], scalar1=rinv[:],
                                    scalar2=None, op0=ALU.mult)

            # ---- func expert combine (natural) ----
            tanh_agg = psml.tile([128, D], dt.float32, tag="tagg")
            nc.scalar.activation(tanh_agg[:], Sagg, AF.Tanh)
            z_nat = psml.tile([128, D], dt.float32, tag="znat")
            nc.vector.transpose(z_nat[:], zT[:])
            d2 = psml.tile([128, D], dt.float32, tag="d2")
            nc.vector.tensor_tensor(out=d2[:], in0=tanh_agg[:], in1=csn[:], op=ALU.subtract)
            f1 = psml.tile([128, D], dt.float32, tag="f1")
            nc.vector.tensor_tensor(out=f1[:], in0=z_nat[:], in1=d2[:], op=ALU.mult)
            func_nat = psml.tile([128, D], dt.float32, tag="func")
            nc.vector.tensor_tensor(out=func_nat[:], in0=f1[:], in1=csn[:], op=ALU.add)

            # ---- experts back to natural + weighted combine ----
            local_nat = psml.tile([128, D], dt.float32, tag="locnat")
            nc.vector.transpose(local_nat[:], localT[:])
            dist_nat = psml.tile([128, D], dt.float32, tag="distnat")
            nc.vector.transpose(dist_nat[:], xcur[:])

            acc1 = psml.tile([128, D], dt.float32, tag="acc1")
            nc.vector.tensor_scalar(out=acc1[:], in0=local_nat[:],
                                    scalar1=gts[:, 0:1], scalar2=None, op0=ALU.mult)
            acc2 = psml.tile([128, D], dt.float32, tag="acc2")
            nc.vector.scalar_tensor_tensor(out=acc2[:], in0=func_nat[:], scalar=gts[:, 1:2],
                                           in1=acc1[:], op0=ALU.mult, op1=ALU.add)
            acc3 = psml.tile([128, D], dt.float32, tag="acc3")
            nc.vector.scalar_tensor_tensor(out=acc3[:], in0=dist_nat[:], scalar=gts[:, 2:3],
                                           in1=acc2[:], op0=ALU.mult, op1=ALU.add)

            nc.sync.dma_start(a_out[rows, :], acc3[:])


# ---------------------------------------------------------------------------
# host staging
# ---------------------------------------------------------------------------

def stage_inputs(inputs, bs=BS, ct=CT):
    """Returns (in_maps, weights_dict) for run_bass_kernel_spmd."""
    nt = bs // ct
    cs = np.asarray(inputs["current_state"], np.float32)
    nb = np.asarray(inputs["neighbor_states"], np.float32)
    tiers = np.asarray(inputs["tier_ids"], np.int32)

    f32 = np.float32
    W_local = np.asarray(inputs["W_local"], f32)
    W_msg = np.asarray(inputs["W_msg"], f32)
    W_upd = np.asarray(inputs["W_upd"], f32)
    W_cnf = np.asarray(inputs["W_cnf"], f32)
    W_g1 = np.asarray(inputs["W_g1"], f32)
    W_g2 = np.asarray(inputs["W_g2"], f32)
    b_msg = np.asarray(inputs["b_msg"], f32)
    b_local = np.asarray(inputs["b_local"], f32)
    b_upd = np.asarray(inputs["b_upd"], f32)
    b_cnf = np.asarray(inputs["b_cnf"], f32)
    b_g1 = np.asarray(inputs["b_g1"], f32)
    b_g2 = np.asarray(inputs["b_g2"], f32)

    eye4 = np.eye(4, dtype=f32)

    def kron4(w):
        return np.kron(eye4, w)

    wparts = {
        "W4msg": kron4(W_msg),
        "Wl_t": kron4(W_local[:D]), "Wl_b": kron4(W_local[D:]),
        "Wu_t": kron4(W_upd[:D]), "Wu_b": kron4(W_upd[D:]),
        "Wc_t": kron4(W_cnf[:D]), "Wc_b": kron4(W_cnf[D:]),
        "Wg1_t": kron4(W_g1[:D]), "Wg1_b": kron4(W_g1[D:] / K),
        "I128": np.eye(128, dtype=f32),
    }
    wc = np.zeros((128, WC_COLS), f32)
    for name in _WSLOTS:
        wc[:, _wslot(name):_wslot(name) + 128] = wparts[name]
    for g in range(3):
        wc[:, 128 * len(_WSLOTS) + 32 * g: 128 * len(_WSLOTS) + 32 * g + 32] = W_g2[:, g][None, :]
    wc = wc.astype(bf16)

    bcq = np.zeros((128, BC_COLS), f32)
    bcq[:, 0] = np.tile(b_msg, 4)
    bcq[:, 1] = np.tile(b_local, 4)
    bcq[:, 2] = np.tile(b_upd, 4)
    bcq[:, 3] = np.tile(b_cnf, 4)
    bcq[:, 4] = np.tile(b_g1, 4)
    bcq[:, 5:8] = b_g2[None, :]

    in_maps = []
    for c in range(N_CORES):
        rs = slice(c * bs, (c + 1) * bs)
        nb_c = nb[rs]
        cs_c = cs[rs]
        tr_c = tiers[rs]

        nbn = nb_c.reshape(bs, K * D).astype(bf16)

        cs4 = cs_c.reshape(nt, 4, 32, D).transpose(0, 1, 3, 2)  # [t, a, d, c]
        cst = cs4.reshape(nt, 128, 32).transpose(1, 0, 2).reshape(128, nt * 32).astype(bf16)

        mA = (tr_c >= 1)
        mB = (tr_c == 2)
        m1 = (tr_c == 1)
        cnt0 = (tr_c == 0).sum(-1).astype(f32)
        cnt1 = m1.sum(-1).astype(f32)
        cnt2 = mB.sum(-1).astype(f32)
        wfun = m1.astype(f32) / np.maximum(cnt1, 1.0)[:, None]
        msk = np.zeros((bs, 80), f32)
        msk[:, 0:K] = mA
        msk[:, 26:26 + K] = mB
        msk[:, 52:52 + K] = wfun
        msk = msk.astype(bf16)

        scl = np.zeros((bs, 4), f32)
        scl[:, 0] = 1.0 / np.maximum(cnt0, 1.0)
        scl[:, 1] = 1.0 / np.maximum(cnt2, 1.0)

        in_maps.append({
            "nbn": nbn, "csn": cs_c.astype(f32), "cst": cst,
            "msk": msk, "scl": scl, "wc": wc, "bc": bcq,
        })
    return in_maps


_PROGRAM_CACHE = {}


def kernel(**inputs):
    from concourse.bass_utils import run_bass_kernel_spmd

    key = (BS, CT)
    if key not in _PROGRAM_CACHE:
        _PROGRAM_CACHE[key] = build_program(BS, CT)
    nc = _PROGRAM_CACHE[key]

    in_maps = stage_inputs(inputs, BS, CT)
    res = run_bass_kernel_spmd(nc, in_maps, core_ids=list(range(N_CORES)))
    out = np.concatenate([r["out"] for r in res.results], axis=0)
    return out.astype(np.float32)
